# revision 1
# baseline (speedup 1.0000x reference)
"""Trainium2 Bass kernel for nn_CacheAugmentation.

Strategy (8 NeuronCores, no collectives — measured collective BW on this stack
is far too low for multi-MB exchanges):
  - Shard the 2048 query rows 8 ways (256 rows/core); each core runs the full
    two-tier cache attention for its rows.
  - Cache-side projections (K = keys@Wk, V_hot = values@Wv, V_cold =
    (values@Wc+bc)@Wd) are replicated per core, streamed in 512-entry chunks
    flash-attention style with per-tier softmax.
  - Scores kept in [cache, query] layout: the exp bias (age/access) becomes a
    per-partition ACT bias, attn@V needs no transposes, and the softmax
    denominator is folded into the attn@V matmul via a ones column (M=65).
  - Host-side preprocessing (free for the device): transpose keys/values/x,
    cast operands to fp16, fold bv/bd/bo into one output-constant vector
    cvec = (bv+bd)@Wo + 2*bo (softmax weights sum to 1, so the value bias
    passes through attention unchanged); bk dropped entirely (it adds a
    per-query constant to scores, which softmax cancels).
  - fp16 matmuls (full PE rate; fp32r is rejected by walrus codegen and fp32
    runs at quarter rate), fp32 accumulation in PSUM; the final out@Wo runs
    in fp32. End-to-end max error vs fp64 measured ~7e-4 of output scale.

Hardware constraints discovered on this TRN2 + walrus build (load-bearing):
  - Only ONE semaphore wait per instruction survives codegen; split_waits()
    moves extras onto same-engine NoOps (~4us modeled cost).
  - Any change of matmul operand base_partition (0<->64, either direction,
    even across separate PSUM banks/groups, even with a PE drain between)
    raises NRT_EXEC_UNIT_UNRECOVERABLE. Hence every matmul here runs at
    base 0: K/Q live in [64-partition, head-major] tiles, and the odd-head
    halves of projection outputs (PSUM rows 64-127) are relocated via
    DVE-copy -> staging SBUF -> SBUF DMA (the only partition-shifting path;
    DMA cannot read PSUM). This also forecloses tile_position row-packing
    of the K=64 score matmuls (~27us PE left on the table).
  - matmul start=True zeroes the full 2KB PSUM bank, so sub-bank
    accumulation regions share exactly one start/stop per bank.
Cost-model timeline: ~440us/core end-to-end vs ~330us PE-busy; buffer-count
sweeps (vext/kt/kraw/epool/PSUM pools) all model within +-1%, so the
schedule is at the local optimum of the available knobs.
"""
import sys

if "/opt/trn_rl_repo" not in sys.path:
    sys.path.insert(0, "/opt/trn_rl_repo")

import numpy as np

import concourse.bass as bass
import concourse.mybir as mybir
import concourse.tile as tile

F32 = mybir.dt.float32
F16 = mybir.dt.float16
AF = mybir.ActivationFunctionType

B, S, HID, NH, CACHE = 2, 1024, 1024, 16, 4096
HD = HID // NH          # 64
HOT = CACHE // 4        # 1024
COLD = CACHE - HOT      # 3072
COMP = HID // 2         # 512
EPS = 1e-5
NCORES = 8
SQ = B * S // NCORES    # 256 query rows per core
CH = 512                # cache chunk
NCB = CH // 128         # c-blocks per chunk (4)
NCH = CACHE // CH       # 8 chunks
HOT_NCH = HOT // CH     # 2 hot chunks


def split_waits(nc, max_waits=1):
    """walrus in this env rejects >1 sync-wait per instruction; move excess
    waits onto NoOps inserted just before, on the same engine (same-engine
    instructions execute in order, so semantics are preserved)."""
    n_split = 0
    for func in nc.m.functions:
        for blk in func.blocks:
            new = []
            for ins in blk.instructions:
                si = ins.sync_info
                if si is not None and si.on_wait and len(si.on_wait) > max_waits:
                    waits = list(si.on_wait)
                    idx = 0
                    while len(waits) > max_waits:
                        chunk, waits = waits[:max_waits], waits[max_waits:]
                        nop = mybir.InstNoOp(
                            name=f"{ins.name}-waitsplit{idx}",
                            ins=[], outs=[],
                            sync_info=mybir.SyncInfo(on_wait=chunk, on_update=[]),
                        )
                        nop.engine = ins.engine
                        new.append(nop)
                        idx += 1
                        n_split += 1
                    si.on_wait = waits
                new.append(ins)
            blk.instructions = new
    return n_split


BUFS = {}


def build_nc(split_for_hw=True):
    _b = lambda k, d: BUFS.get(k, d)
    nc = bass.Bass(trn_type="TRN2")

    # ---- DRAM I/O ----
    xT = nc.dram_tensor("xT_shard", [HID, SQ], F16, kind="ExternalInput")
    keysT = nc.dram_tensor("keysT", [HID, CACHE], F16, kind="ExternalInput")
    vT_hot = nc.dram_tensor("vT_hot", [HID, HOT], F16, kind="ExternalInput")
    vT_cold = nc.dram_tensor("vT_cold", [HID, COLD], F16, kind="ExternalInput")
    Wq = nc.dram_tensor("Wq", [HID, HID], F16, kind="ExternalInput")
    Wk = nc.dram_tensor("Wk", [HID, HID], F16, kind="ExternalInput")
    Wv = nc.dram_tensor("Wv", [HID, HID], F16, kind="ExternalInput")
    Wc = nc.dram_tensor("Wc", [HID, COMP], F16, kind="ExternalInput")
    Wd = nc.dram_tensor("Wd", [COMP, HID], F16, kind="ExternalInput")
    Wo = nc.dram_tensor("Wo", [HID, HID], F32, kind="ExternalInput")
    bq = nc.dram_tensor("bq", [HID], F32, kind="ExternalInput")
    bc = nc.dram_tensor("bc", [COMP], F32, kind="ExternalInput")
    biasc = nc.dram_tensor("biasc", [CACHE], F32, kind="ExternalInput")
    cvec = nc.dram_tensor("cvec", [HID], F32, kind="ExternalInput")
    gamma = nc.dram_tensor("gamma", [HID], F32, kind="ExternalInput")
    beta = nc.dram_tensor("beta", [HID], F32, kind="ExternalInput")
    y_out = nc.dram_tensor("y_shard", [SQ, HID], F32, kind="ExternalOutput")

    NB = CACHE // 128  # 32 global cache blocks

    from contextlib import ExitStack
    with tile.TileContext(nc) as tc, ExitStack() as ctx:
        constp = ctx.enter_context(tc.tile_pool(name="const", bufs=1))
        vwp = ctx.enter_context(tc.tile_pool(name="vw", bufs=1))
        wrowp = ctx.enter_context(tc.tile_pool(name="wrow", bufs=_b("wrow", 2)))
        krawp = ctx.enter_context(tc.tile_pool(name="kraw", bufs=_b("kraw", 2)))
        kprojp = ctx.enter_context(tc.tile_pool(name="kproj", bufs=_b("kproj", 2)))
        vextp = ctx.enter_context(tc.tile_pool(name="vextp", bufs=_b("vextp", 1)))
        ctp = ctx.enter_context(tc.tile_pool(name="ctp", bufs=_b("ctp", 1)))
        epool = ctx.enter_context(tc.tile_pool(name="epool", bufs=_b("epool", 5)))
        ypool = ctx.enter_context(tc.tile_pool(name="ypool", bufs=2))
        gbpool = ctx.enter_context(tc.tile_pool(name="gbpool", bufs=1))
        lbcp = ctx.enter_context(tc.tile_pool(name="lbcp", bufs=1))
        stagep = ctx.enter_context(tc.tile_pool(name="stage", bufs=_b("stage", 2)))
        dramp = ctx.enter_context(tc.tile_pool(name="dram", bufs=1, space="DRAM"))
        pproj = ctx.enter_context(tc.tile_pool(name="pproj", bufs=_b("pproj", 2), space="PSUM"))
        pst = ctx.enter_context(tc.tile_pool(name="pst", bufs=_b("pst", 2), space="PSUM"))
        pacc = ctx.enter_context(tc.tile_pool(name="pacc", bufs=_b("pacc", 2), space="PSUM"))
        if True:
            # ---- resident constants ----
            wk_sb = constp.tile([128, 8, HID], F16, tag="wk")
            nc.sync.dma_start(wk_sb, Wk[:, :].rearrange("(ib p) o -> p ib o", p=128))
            qT_sb = constp.tile([64, NH, SQ], F16, tag="qT")
            biasc_sb = constp.tile([128, NB], F32, tag="biasc")
            nc.sync.dma_start(biasc_sb, biasc[:].rearrange("(g p) -> p g", p=128))
            bq_sb = constp.tile([128, 8], F32, tag="bq")
            nc.sync.dma_start(bq_sb, bq[:].rearrange("(ob p) -> p ob", p=128))
            bc_sb = constp.tile([128, 4], F32, tag="bc")
            nc.sync.dma_start(bc_sb, bc[:].rearrange("(ob p) -> p ob", p=128))
            ones_sb = constp.tile([1, 128], F32, tag="ones")
            nc.vector.memset(ones_sb, 1.0)
            cvec_sb = constp.tile([1, HID], F32, tag="cvec")
            nc.sync.dma_start(cvec_sb, cvec[:].unsqueeze(0))
            eps_sb = constp.tile([128, 1], F32, tag="eps")
            nc.vector.memset(eps_sb, EPS)
            acc_sb = constp.tile([128, NH, SQ], F32, tag="acc")
            aoT_sb = constp.tile([128, 8, SQ], F32, tag="aoT")
            xT_sb = constp.tile([128, 8, SQ], F16, tag="xT")
            nc.sync.dma_start(xT_sb, xT[:, :].rearrange("(ib p) s -> p ib s", p=128))
            lbc_sb = lbcp.tile([64, NH // 2, SQ], F32, tag="lbc")
            lscr = dramp.tile([1, NH * SQ], F32, tag="lscr")

            # ---- q projection: qT[o, s] = Wq.T @ xT (+bq at eviction) ----
            qps = [pst.tile([128, 4 * SQ], F32, tag="st", name=f"qps{i}") for i in range(2)]
            for ib in range(8):
                wq_strip = wrowp.tile([128, HID], F16, tag="wq")
                nc.sync.dma_start(wq_strip, Wq[ib * 128:(ib + 1) * 128, :])
                for ob in range(8):
                    nc.tensor.matmul(
                        qps[ob // 4][:, (ob % 4) * SQ:(ob % 4 + 1) * SQ],
                        wq_strip[:, ob * 128:(ob + 1) * 128],
                        xT_sb[:, ib, :],
                        start=(ib == 0 and ob % 2 == 0),
                        stop=(ib == 7 and ob % 2 == 1),
                    )
            for ob in range(8):
                src_ps = qps[ob // 4][:, (ob % 4) * SQ:(ob % 4 + 1) * SQ]
                nc.scalar.activation(
                    qT_sb[0:64, 2 * ob, :], src_ps[0:64, :],
                    AF.Identity, bias=bq_sb[0:64, ob:ob + 1], scale=1.0,
                )
                stg = stagep.tile([128, SQ], F16, tag="stg")
                nc.scalar.activation(
                    stg[64:128, :], src_ps[64:128, :],
                    AF.Identity, bias=bq_sb[64:128, ob:ob + 1], scale=1.0,
                )
                nc.sync.dma_start(qT_sb[0:64, 2 * ob + 1, :], stg[64:128, :])

            # ---- cache chunk loop ----
            wv_view = None
            wc_view = None
            wd_view = None
            for c in range(NCH):
                hot = c < HOT_NCH
                c0 = c * CH
                if c == 0:
                    vw_flat = vwp.tile([128, 8 * HID], F16, tag="vw")
                    wv_view = vw_flat.rearrange("p (ib o) -> p ib o", ib=8)
                    nc.sync.dma_start(
                        wv_view, Wv[:, :].rearrange("(ib p) o -> p ib o", p=128))
                if c == HOT_NCH:
                    vw_flat = vwp.tile([128, 8 * HID], F16, tag="vw")
                    wc_view = vw_flat[:, 0:8 * COMP].rearrange(
                        "p (ib o) -> p ib o", ib=8)
                    nc.sync.dma_start(
                        wc_view, Wc[:, :].rearrange("(ib p) o -> p ib o", p=128))
                    wd_view = vw_flat[:, 8 * COMP:8 * COMP + 4 * HID].rearrange(
                        "p (ib o) -> p ib o", ib=4)
                    nc.sync.dma_start(
                        wd_view, Wd[:, :].rearrange("(ib p) o -> p ib o", p=128))

                ktc = krawp.tile([128, 8, CH], F16, tag="ktc")
                nc.sync.dma_start(
                    ktc, keysT[:, c0:c0 + CH].rearrange("(ib p) c -> p ib c", p=128))
                vtc = krawp.tile([128, 8, CH], F16, tag="vtc")
                vsrc = vT_hot[:, c0:c0 + CH] if hot else \
                    vT_cold[:, c0 - HOT:c0 - HOT + CH]
                nc.sync.dma_start(
                    vtc, vsrc.rearrange("(ib p) c -> p ib c", p=128))

                # -- K projection: kT[o, c] = Wk.T @ keysT_chunk --
                kt = kprojp.tile([64, NH, CH], F16, tag="kt")
                for ob in range(8):
                    ps = pproj.tile([128, 512], F32, tag="pp")
                    for ib in range(8):
                        nc.tensor.matmul(
                            ps,
                            wk_sb[:, ib, ob * 128:(ob + 1) * 128],
                            ktc[:, ib, :],
                            start=(ib == 0), stop=(ib == 7),
                        )
                    if ob % 2 == 0:
                        nc.scalar.copy(kt[0:64, ob, :], ps[0:64, :])
                        stg = stagep.tile([128, CH], F16, tag="stgk")
                        nc.vector.tensor_copy(stg[64:128, :], ps[64:128, :])
                    else:
                        nc.vector.tensor_copy(kt[0:64, ob, :], ps[0:64, :])
                        stg = stagep.tile([128, CH], F16, tag="stgk")
                        nc.scalar.copy(stg[64:128, :], ps[64:128, :])
                    nc.sync.dma_start(kt[0:64, ob + 8, :], stg[64:128, :])

                # -- V projection into vext [c, 16*(64+1)] (ones col per head) --
                vext_t = vextp.tile([128, NCB, NH * (HD + 1)], F16, tag="vext")
                if hot:
                    for cb in range(NCB):
                        for oc in range(2):
                            ps = pproj.tile([128, 512], F32, tag="pp")
                            for ib in range(8):
                                nc.tensor.matmul(
                                    ps,
                                    vtc[:, ib, cb * 128:(cb + 1) * 128],
                                    wv_view[:, ib, oc * 512:(oc + 1) * 512],
                                    start=(ib == 0), stop=(ib == 7),
                                )
                            dst = vext_t[:, cb, oc * 520:(oc + 1) * 520].rearrange(
                                "p (h e) -> p h e", h=8)[:, :, 0:HD]
                            nc.vector.tensor_copy(
                                dst, ps[:, :].rearrange("p (h e) -> p h e", e=HD))
                else:
                    # compress: cT[o', c] = Wc.T @ valuesT_chunk (+bc)
                    ct = ctp.tile([128, 4, CH], F16, tag="ct")
                    for obq in range(4):
                        ps = pproj.tile([128, 512], F32, tag="pp")
                        for ib in range(8):
                            nc.tensor.matmul(
                                ps,
                                wc_view[:, ib, obq * 128:(obq + 1) * 128],
                                vtc[:, ib, :],
                                start=(ib == 0), stop=(ib == 7),
                            )
                        nc.scalar.activation(
                            ct[:, obq, :], ps,
                            AF.Identity, bias=bc_sb[:, obq:obq + 1], scale=1.0,
                        )
                    # decompress: v[c, o] = cT.T @ Wd
                    for cb in range(NCB):
                        for oc in range(2):
                            ps = pproj.tile([128, 512], F32, tag="pp")
                            for ibq in range(4):
                                nc.tensor.matmul(
                                    ps,
                                    ct[:, ibq, cb * 128:(cb + 1) * 128],
                                    wd_view[:, ibq, oc * 512:(oc + 1) * 512],
                                    start=(ibq == 0), stop=(ibq == 3),
                                )
                            dst = vext_t[:, cb, oc * 520:(oc + 1) * 520].rearrange(
                                "p (h e) -> p h e", h=8)[:, :, 0:HD]
                            nc.vector.tensor_copy(
                                dst, ps[:, :].rearrange("p (h e) -> p h e", e=HD))
                nc.vector.memset(
                    vext_t.rearrange("p cb (h e) -> p cb h e", e=HD + 1)[:, :, :, HD:HD + 1],
                    1.0)

                # -- attention for this chunk --
                for hg in range(4):
                    e_ts = []
                    for cb in range(NCB):
                        g = c * NCB + cb
                        stp = pst.tile([128, 4 * SQ], F32, tag="st")
                        for hh in range(4):
                            h = hg * 4 + hh
                            ki = (h // 2) if h % 2 == 0 else (h // 2 + 8)
                            nc.tensor.matmul(
                                stp[:, hh * SQ:(hh + 1) * SQ],
                                kt[0:64, ki, cb * 128:(cb + 1) * 128],
                                qT_sb[0:64, h, :],
                                start=(hh % 2 == 0), stop=(hh % 2 == 1),
                            )
                        e_t = epool.tile([128, 4, SQ], F16, tag="e")
                        nc.scalar.activation(
                            e_t, stp[:, :].rearrange("p (a b) -> p a b", a=4),
                            AF.Exp, bias=biasc_sb[:, g:g + 1], scale=0.125,
                        )
                        e_ts.append(e_t)
                    for pr in range(2):
                        pa = pacc.tile([128, 2 * SQ], F32, tag="pa")
                        for cb in range(NCB):
                            for sub in range(2):
                                h = hg * 4 + pr * 2 + sub
                                nc.tensor.matmul(
                                    pa[0:65, sub * SQ:(sub + 1) * SQ],
                                    vext_t[:, cb, h * 65:h * 65 + 65],
                                    e_ts[cb][:, pr * 2 + sub, :],
                                    start=(cb == 0 and sub == 0),
                                    stop=(cb == NCB - 1 and sub == 1),
                                )
                        h0 = hg * 4 + pr * 2
                        dst = acc_sb[0:65, h0:h0 + 2, :]
                        src = pa[0:65, :].rearrange("p (a b) -> p a b", a=2)
                        if c == 0 or c == HOT_NCH:
                            nc.vector.tensor_copy(dst, src)
                        else:
                            nc.vector.tensor_add(dst, dst, src)

                # -- per-tier softmax division at tier end --
                if c == HOT_NCH - 1 or c == NCH - 1:
                    first_tier = c == HOT_NCH - 1
                    nc.vector.reciprocal(acc_sb[64:65, :, :], acc_sb[64:65, :, :])
                    nc.sync.dma_start(
                        lscr[0:1, :],
                        acc_sb[64:65, :, :].rearrange("p a b -> p (a b)"))
                    for h in range(NH):
                        if h % 8 == 0:
                            nc.sync.dma_start(
                                lbc_sb,
                                lscr[0:1, (h // 8) * 8 * SQ:(h // 8 + 1) * 8 * SQ]
                                .to_broadcast([64, 8 * SQ]).rearrange(
                                    "p (a b) -> p a b", a=8))
                        num = acc_sb[0:64, h, :]
                        rc = lbc_sb[0:64, h % 8, :]
                        dst = aoT_sb[(h % 2) * 64:(h % 2) * 64 + 64, h // 2, :]
                        if first_tier:
                            if h % 2 == 0:
                                nc.vector.tensor_mul(dst, num, rc)
                            else:
                                tmp = epool.tile([128, 4, SQ], F32, tag="dtmp", bufs=2)
                                nc.vector.tensor_mul(tmp[0:64, 0, :], num, rc)
                                nc.sync.dma_start(dst, tmp[0:64, 0, :])
                        else:
                            tmp = epool.tile([128, 4, SQ], F32, tag="dtmp", bufs=2)
                            nc.vector.tensor_mul(tmp[0:64, 0, :], num, rc)
                            if h % 2 == 0:
                                nc.vector.tensor_add(dst, dst, tmp[0:64, 0, :])
                            else:
                                tmp2 = epool.tile([128, 4, SQ], F32, tag="dtmp", bufs=2)
                                nc.sync.dma_start(
                                    tmp2[64:128, 0, :], tmp[0:64, 0, :])
                                nc.vector.tensor_add(dst, dst, tmp2[64:128, 0, :])

            # ---- output projection y = aoT.T @ Wo + cvec (fp32), layernorm ----
            yps = [pst.tile([128, 4 * SQ], F32, tag="st", name=f"yps{i}") for i in range(2)]
            for ib in range(8):
                wo_strip = wrowp.tile([128, HID], F32, tag="wo")
                nc.sync.dma_start(wo_strip, Wo[ib * 128:(ib + 1) * 128, :])
                for sblk in range(2):
                    for oc in range(2):
                        nc.tensor.matmul(
                            yps[sblk][:, oc * 512:(oc + 1) * 512],
                            aoT_sb[:, ib, sblk * 128:(sblk + 1) * 128],
                            wo_strip[:, oc * 512:(oc + 1) * 512],
                            start=(ib == 0), stop=False,
                        )
            for sblk in range(2):
                for oc in range(2):
                    nc.tensor.matmul(
                        yps[sblk][:, oc * 512:(oc + 1) * 512],
                        ones_sb[0:1, 0:128],
                        cvec_sb[0:1, oc * 512:(oc + 1) * 512],
                        start=False, stop=True,
                    )

            gb_t = gbpool.tile([128, 2 * HID], F32, tag="gb")
            nc.sync.dma_start(
                gb_t[:, 0:HID], gamma[:].unsqueeze(0).to_broadcast([128, HID]))
            nc.sync.dma_start(
                gb_t[:, HID:2 * HID], beta[:].unsqueeze(0).to_broadcast([128, HID]))

            for sblk in range(2):
                y_sb = ypool.tile([128, HID], F32, tag="y")
                nc.scalar.copy(y_sb, yps[sblk][:, :])
                stats = ypool.tile([128, 2, 6], F32, tag="stats")
                for sub in range(2):
                    nc.vector.bn_stats(
                        stats[:, sub, :], y_sb[:, sub * 512:(sub + 1) * 512])
                mv = ypool.tile([128, 2], F32, tag="mv")
                nc.vector.bn_aggr(mv, stats)
                rstd = ypool.tile([128, 1], F32, tag="rstd")
                nc.scalar.activation(
                    rstd, mv[:, 1:2], AF.Sqrt, bias=eps_sb[:, 0:1], scale=1.0)
                nc.vector.reciprocal(rstd, rstd)
                nc.vector.tensor_scalar(
                    y_sb, y_sb, mv[:, 0:1], rstd,
                    op0=mybir.AluOpType.subtract, op1=mybir.AluOpType.mult)
                nc.vector.tensor_mul(y_sb, y_sb, gb_t[:, 0:HID])
                nc.vector.tensor_add(y_sb, y_sb, gb_t[:, HID:2 * HID])
                nc.sync.dma_start(y_out[sblk * 128:(sblk + 1) * 128, :], y_sb)

    if split_for_hw:
        split_waits(nc)
    return nc


_NC_CACHE = None


def _get_nc():
    global _NC_CACHE
    if _NC_CACHE is None:
        _NC_CACHE = build_nc()
    return _NC_CACHE


def _prep_inputs(inputs):
    f32 = lambda a: np.ascontiguousarray(np.asarray(a, dtype=np.float32))
    f16 = lambda a: np.ascontiguousarray(np.asarray(a, dtype=np.float32).astype(np.float16))
    x = np.asarray(inputs["inputs"], dtype=np.float32).reshape(B * S, HID)
    keys = np.concatenate(
        [np.asarray(inputs["hot_keys"], np.float32),
         np.asarray(inputs["cold_keys"], np.float32)], axis=0)
    biasc = np.concatenate([
        -0.1 * f32(inputs["hot_age"]) + 0.05 * f32(inputs["hot_access"]),
        -0.1 * f32(inputs["cold_age"]) + 0.05 * f32(inputs["cold_access"]),
    ])
    bv = f32(inputs["bv"])
    bd = f32(inputs["bd"])
    bo = f32(inputs["bo"])
    Wo = f32(inputs["Wo"])
    cvec = (bv + bd) @ Wo + 2.0 * bo
    shared = {
        "keysT": f16(keys.T),
        "vT_hot": f16(np.asarray(inputs["hot_values"], np.float32).T),
        "vT_cold": f16(np.asarray(inputs["cold_values"], np.float32).T),
        "Wq": f16(inputs["Wq"]),
        "Wk": f16(inputs["Wk"]),
        "Wv": f16(inputs["Wv"]),
        "Wc": f16(inputs["Wc"]),
        "Wd": f16(inputs["Wd"]),
        "Wo": Wo,
        "bq": f32(inputs["bq"]),
        "bc": f32(inputs["bc"]),
        "biasc": np.ascontiguousarray(biasc.astype(np.float32)),
        "cvec": np.ascontiguousarray(cvec.astype(np.float32)),
        "gamma": f32(inputs["gamma"]),
        "beta": f32(inputs["beta"]),
    }
    xT16 = np.asarray(x.T, np.float32).astype(np.float16)
    in_maps = []
    for i in range(NCORES):
        m = dict(shared)
        m["xT_shard"] = np.ascontiguousarray(xT16[:, i * SQ:(i + 1) * SQ])
        in_maps.append(m)
    return in_maps


def _run(inputs, trace=False):
    from concourse.bass_utils import run_bass_kernel_spmd

    nc = _get_nc()
    in_maps = _prep_inputs(inputs)
    res = run_bass_kernel_spmd(
        nc, in_maps, core_ids=list(range(NCORES)), trace=trace)
    y = np.concatenate(
        [res.results[i]["y_shard"] for i in range(NCORES)], axis=0)
    return y.reshape(B, S, HID), res


def kernel(**inputs):
    y, _ = _run(inputs, trace=False)
    return y


def make_test_inputs(seed=0):
    rng = np.random.default_rng(seed)
    std = 0.02
    return {
        "inputs": rng.standard_normal((B, S, HID)).astype(np.float32),
        "hot_keys": (std * rng.standard_normal((HOT, HID))).astype(np.float32),
        "hot_values": (std * rng.standard_normal((HOT, HID))).astype(np.float32),
        "hot_age": np.abs(rng.standard_normal(HOT)).astype(np.float32),
        "hot_access": np.abs(rng.standard_normal(HOT)).astype(np.float32),
        "cold_keys": (std * rng.standard_normal((COLD, HID))).astype(np.float32),
        "cold_values": (std * rng.standard_normal((COLD, HID))).astype(np.float32),
        "cold_age": np.abs(rng.standard_normal(COLD)).astype(np.float32),
        "cold_access": np.abs(rng.standard_normal(COLD)).astype(np.float32),
        "Wq": (std * rng.standard_normal((HID, HID))).astype(np.float32),
        "bq": (0.01 * rng.standard_normal(HID)).astype(np.float32),
        "Wk": (std * rng.standard_normal((HID, HID))).astype(np.float32),
        "bk": (0.01 * rng.standard_normal(HID)).astype(np.float32),
        "Wv": (std * rng.standard_normal((HID, HID))).astype(np.float32),
        "bv": (0.01 * rng.standard_normal(HID)).astype(np.float32),
        "Wo": (std * rng.standard_normal((HID, HID))).astype(np.float32),
        "bo": (0.01 * rng.standard_normal(HID)).astype(np.float32),
        "Wc": ((1.0 / np.sqrt(HID)) * rng.standard_normal((HID, COMP))).astype(np.float32),
        "bc": (0.01 * rng.standard_normal(COMP)).astype(np.float32),
        "Wd": ((1.0 / np.sqrt(COMP)) * rng.standard_normal((COMP, HID))).astype(np.float32),
        "bd": (0.01 * rng.standard_normal(HID)).astype(np.float32),
        "gamma": (1.0 + 0.1 * rng.standard_normal(HID)).astype(np.float32),
        "beta": (0.1 * rng.standard_normal(HID)).astype(np.float32),
    }


def np_reference(inp):
    x = np.asarray(inp["inputs"], np.float64).reshape(B * S, HID)
    q = x @ inp["Wq"] + inp["bq"]
    keys = np.concatenate([inp["hot_keys"], inp["cold_keys"]]).astype(np.float64)
    k = keys @ inp["Wk"] + inp["bk"]
    hot_v = inp["hot_values"].astype(np.float64) @ inp["Wv"] + inp["bv"]
    cold_v = (inp["cold_values"].astype(np.float64) @ inp["Wc"] + inp["bc"]) \
        @ inp["Wd"] + inp["bd"]
    biasv = np.concatenate([
        -0.1 * inp["hot_age"] + 0.05 * inp["hot_access"],
        -0.1 * inp["cold_age"] + 0.05 * inp["cold_access"]]).astype(np.float64)
    qh = q.reshape(B * S, NH, HD)
    kh = k.reshape(CACHE, NH, HD)
    out = np.zeros((B * S, NH, HD))
    for lo, hi, v in [(0, HOT, hot_v), (HOT, CACHE, cold_v)]:
        sc = np.einsum("snd,cnd->snc", qh, kh[lo:hi]) / np.sqrt(HD)
        sc = sc + biasv[lo:hi][None, None, :]
        a = np.exp(sc)
        a /= a.sum(-1, keepdims=True)
        out += np.einsum("snc,cnd->snd", a, v.reshape(hi - lo, NH, HD))
    xx = out.reshape(B * S, HID) @ inp["Wo"] + 2 * inp["bo"]
    mu = xx.mean(-1, keepdims=True)
    var = ((xx - mu) ** 2).mean(-1, keepdims=True)
    y = (xx - mu) / np.sqrt(var + EPS) * inp["gamma"] + inp["beta"]
    return y.reshape(B, S, HID)


if __name__ == "__main__":
    # single-core CoreSim smoke test against the numpy reference
    from concourse.bass_interp import CoreSim

    inputs = make_test_inputs()
    expected = np_reference(inputs)

    nc = build_nc(split_for_hw=False)
    in_maps = _prep_inputs(inputs)
    sim = CoreSim(nc)
    for kname, v in in_maps[0].items():
        sim.tensor(kname)[:] = v
    sim.simulate(check_with_hw=False)
    got = np.array(sim.tensor("y_shard"))
    exp0 = expected.reshape(B * S, HID)[0:SQ]
    err = np.abs(got - exp0)
    denom = np.abs(exp0).max()
    print(f"core0 absmax_err={err.max():.3e} relmax={err.max() / denom:.3e} "
          f"mean={err.mean():.3e}")



# revision 4
# speedup vs baseline: 54.7395x; 54.7395x over previous
"""Trainium2 Bass kernel for nn_CacheAugmentation.

Strategy (8 NeuronCores, no collectives — measured collective BW on this stack
is far too low for multi-MB exchanges):
  - Shard the 2048 query rows 8 ways (256 rows/core); each core runs the full
    two-tier cache attention for its rows.
  - Cache-side projections (K = keys@Wk, V_hot = values@Wv, V_cold =
    (values@Wc+bc)@Wd) are replicated per core, streamed in 512-entry chunks
    flash-attention style with per-tier softmax.
  - Scores kept in [cache, query] layout: the exp bias (age/access) becomes a
    per-partition ACT bias, attn@V needs no transposes, and the softmax
    denominator is folded into the attn@V matmul via a ones column (M=65).
  - ALL inputs are packed host-side into ONE fp16 DRAM blob (xT shard first,
    then the replicated cache/weight regions). Measured on this axon stack,
    per-call wall time scales with the NUMBER of NEFF I/O tensors (~2 ms per
    argument per call) and not with bytes; 17 inputs -> 1 input removes ~30 ms
    of per-call RPC binding overhead. Small fp32 constants (score bias, bq,
    bc, gamma/beta) are carried in fp16 and upcast on device with one DVE
    copy each; Wo/cvec run in fp16 (the out-proj matmul is 4x faster in fp16
    than the old fp32 version, and fp16 quantization is ~30x below the
    correctness tolerance).
  - Host-side preprocessing (free for the device): transpose keys/values/x,
    cast operands to fp16, fold bv/bd/bo into one output-constant vector
    cvec = (bv+bd)@Wo + 2*bo (softmax weights sum to 1, so the value bias
    passes through attention unchanged); bk dropped entirely (it adds a
    per-query constant to scores, which softmax cancels).
  - fp16 matmuls (full PE rate; fp32r is rejected by walrus codegen and fp32
    runs at quarter rate), fp32 accumulation in PSUM.

build_nc(reps=N) replicates the whole body N times in one NEFF; test.py uses
(wall[reps=4] - wall[reps=1]) / 3 to measure true per-execution device time
independent of the ~10 ms axon per-call RPC overhead.

Hardware constraints discovered on this TRN2 + walrus build (load-bearing):
  - Only ONE semaphore wait per instruction survives codegen; split_waits()
    moves extras onto same-engine NoOps (~4us modeled cost).
  - Any change of matmul operand base_partition (0<->64, either direction,
    even across separate PSUM banks/groups, even with a PE drain between)
    raises NRT_EXEC_UNIT_UNRECOVERABLE. Hence every matmul here runs at
    base 0: K/Q live in [64-partition, head-major] tiles, and the odd-head
    halves of projection outputs (PSUM rows 64-127) are relocated via
    DVE-copy -> staging SBUF -> SBUF DMA (the only partition-shifting path;
    DMA cannot read PSUM). This also forecloses tile_position row-packing
    of the K=64 score matmuls (~27us PE left on the table).
  - matmul start=True zeroes the full 2KB PSUM bank, so sub-bank
    accumulation regions share exactly one start/stop per bank.
"""
import sys

if "/opt/trn_rl_repo" not in sys.path:
    sys.path.insert(0, "/opt/trn_rl_repo")

import numpy as np

import concourse.bass as bass
import concourse.mybir as mybir
import concourse.tile as tile

F32 = mybir.dt.float32
F16 = mybir.dt.float16
AF = mybir.ActivationFunctionType

B, S, HID, NH, CACHE = 2, 1024, 1024, 16, 4096
HD = HID // NH          # 64
HOT = CACHE // 4        # 1024
COLD = CACHE - HOT      # 3072
COMP = HID // 2         # 512
EPS = 1e-5
NCORES = 8
SQ = B * S // NCORES    # 256 query rows per core
CH = 512                # cache chunk
NCB = CH // 128         # c-blocks per chunk (4)
NCH = CACHE // CH       # 8 chunks
HOT_NCH = HOT // CH     # 2 hot chunks

# ---- packed input blob layout (fp16, element offsets) ----
_REGIONS = [
    ("xT", HID * SQ),          # per-core x.T shard [HID, SQ]
    ("keysT", HID * CACHE),    # keys.T [HID, CACHE] (hot then cold)
    ("vT_hot", HID * HOT),     # hot_values.T
    ("vT_cold", HID * COLD),   # cold_values.T
    ("Wq", HID * HID),
    ("Wk", HID * HID),
    ("Wv", HID * HID),
    ("Wc", HID * COMP),
    ("Wd", COMP * HID),
    ("Wo", HID * HID),
    ("bq", HID),
    ("bc", COMP),
    ("biasc", CACHE),          # -0.1*age + 0.05*access
    ("cvec", HID),             # (bv+bd)@Wo + 2*bo
    ("gamma", HID),
    ("beta", HID),             # must stay adjacent to gamma (one DMA)
]
OFF = {}
_o = 0
for _n, _sz in _REGIONS:
    OFF[_n] = (_o, _sz)
    _o += _sz
BLOB_N = _o


def split_waits(nc, max_waits=1):
    """walrus in this env rejects >1 sync-wait per instruction; move excess
    waits onto NoOps inserted just before, on the same engine (same-engine
    instructions execute in order, so semantics are preserved)."""
    n_split = 0
    for func in nc.m.functions:
        for blk in func.blocks:
            new = []
            for ins in blk.instructions:
                si = ins.sync_info
                if si is not None and si.on_wait and len(si.on_wait) > max_waits:
                    waits = list(si.on_wait)
                    idx = 0
                    while len(waits) > max_waits:
                        chunk, waits = waits[:max_waits], waits[max_waits:]
                        nop = mybir.InstNoOp(
                            name=f"{ins.name}-waitsplit{idx}",
                            ins=[], outs=[],
                            sync_info=mybir.SyncInfo(on_wait=chunk, on_update=[]),
                        )
                        nop.engine = ins.engine
                        new.append(nop)
                        idx += 1
                        n_split += 1
                    si.on_wait = waits
                new.append(ins)
            blk.instructions = new
    return n_split


BUFS = {}


def build_nc(split_for_hw=True, reps=1):
    _b = lambda k, d: BUFS.get(k, d)
    nc = bass.Bass(trn_type="TRN2")

    blob = nc.dram_tensor("blob", [BLOB_N], F16, kind="ExternalInput")
    y_out = nc.dram_tensor("y_shard", [SQ, HID], F32, kind="ExternalOutput")

    def reg(name):
        o, n = OFF[name]
        return blob[o:o + n]

    NB = CACHE // 128  # 32 global cache blocks

    from contextlib import ExitStack
    with tile.TileContext(nc) as tc, ExitStack() as ctx:
        constp = ctx.enter_context(tc.tile_pool(name="const", bufs=1))
        vwp = ctx.enter_context(tc.tile_pool(name="vw", bufs=1))
        wrowp = ctx.enter_context(tc.tile_pool(name="wrow", bufs=_b("wrow", 2)))
        krawp = ctx.enter_context(tc.tile_pool(name="kraw", bufs=_b("kraw", 2)))
        kprojp = ctx.enter_context(tc.tile_pool(name="kproj", bufs=_b("kproj", 2)))
        vextp = ctx.enter_context(tc.tile_pool(name="vextp", bufs=_b("vextp", 1)))
        ctp = ctx.enter_context(tc.tile_pool(name="ctp", bufs=_b("ctp", 1)))
        epool = ctx.enter_context(tc.tile_pool(name="epool", bufs=_b("epool", 4)))
        ypool = ctx.enter_context(tc.tile_pool(name="ypool", bufs=2))
        gbpool = ctx.enter_context(tc.tile_pool(name="gbpool", bufs=1))
        lbcp = ctx.enter_context(tc.tile_pool(name="lbcp", bufs=1))
        stagep = ctx.enter_context(tc.tile_pool(name="stage", bufs=_b("stage", 2)))
        dramp = ctx.enter_context(tc.tile_pool(name="dram", bufs=1, space="DRAM"))
        pproj = ctx.enter_context(tc.tile_pool(name="pproj", bufs=_b("pproj", 2), space="PSUM"))
        pst = ctx.enter_context(tc.tile_pool(name="pst", bufs=_b("pst", 2), space="PSUM"))
        pacc = ctx.enter_context(tc.tile_pool(name="pacc", bufs=_b("pacc", 2), space="PSUM"))
        for _rep in range(reps):
            # ---- resident constants ----
            wk_sb = constp.tile([128, 8, HID], F16, tag="wk")
            nc.sync.dma_start(
                wk_sb, reg("Wk").rearrange("(ib p o) -> p ib o", p=128, o=HID))
            qT_sb = constp.tile([64, NH, SQ], F16, tag="qT")
            biasc_st = constp.tile([128, NB], F16, tag="biasc_st")
            nc.sync.dma_start(
                biasc_st, reg("biasc").rearrange("(g p) -> p g", p=128))
            biasc_sb = constp.tile([128, NB], F32, tag="biasc")
            nc.vector.tensor_copy(biasc_sb, biasc_st)
            bqc_st = constp.tile([128, 12], F16, tag="bqc_st")
            nc.sync.dma_start(
                bqc_st[:, 0:8], reg("bq").rearrange("(ob p) -> p ob", p=128))
            nc.sync.dma_start(
                bqc_st[:, 8:12], reg("bc").rearrange("(ob p) -> p ob", p=128))
            bqc_sb = constp.tile([128, 12], F32, tag="bqc")
            nc.vector.tensor_copy(bqc_sb, bqc_st)
            bq_sb = bqc_sb[:, 0:8]
            bc_sb = bqc_sb[:, 8:12]
            ones_sb = constp.tile([1, 128], F16, tag="ones")
            nc.vector.memset(ones_sb, 1.0)
            cvec_sb = constp.tile([1, HID], F16, tag="cvec")
            nc.sync.dma_start(cvec_sb, reg("cvec").unsqueeze(0))
            eps_sb = constp.tile([128, 1], F32, tag="eps")
            nc.vector.memset(eps_sb, EPS)
            acc_sb = constp.tile([128, NH, SQ], F32, tag="acc")
            aoT_sb = constp.tile([128, 8, SQ], F32, tag="aoT")
            aoT16_sb = constp.tile([128, 8, SQ], F16, tag="aoT16")
            xT_sb = constp.tile([128, 8, SQ], F16, tag="xT")
            nc.sync.dma_start(
                xT_sb, reg("xT").rearrange("(ib p s) -> p ib s", p=128, s=SQ))
            lbc_sb = lbcp.tile([64, NH // 2, SQ], F32, tag="lbc")
            lscr = dramp.tile([1, NH * SQ], F32, tag="lscr")

            wq_v = reg("Wq").rearrange("(r o) -> r o", o=HID)
            wo_v = reg("Wo").rearrange("(r o) -> r o", o=HID)
            keys_v = reg("keysT").rearrange("(ib p c) -> p ib c", p=128, c=CACHE)
            vhot_v = reg("vT_hot").rearrange("(ib p c) -> p ib c", p=128, c=HOT)
            vcold_v = reg("vT_cold").rearrange("(ib p c) -> p ib c", p=128, c=COLD)

            # ---- q projection: qT[o, s] = Wq.T @ xT (+bq at eviction) ----
            qps = [pst.tile([128, 4 * SQ], F32, tag="st", name=f"qps{i}") for i in range(2)]
            for ib in range(8):
                wq_strip = wrowp.tile([128, HID], F16, tag="wq")
                nc.sync.dma_start(wq_strip, wq_v[ib * 128:(ib + 1) * 128, :])
                for ob in range(8):
                    nc.tensor.matmul(
                        qps[ob // 4][:, (ob % 4) * SQ:(ob % 4 + 1) * SQ],
                        wq_strip[:, ob * 128:(ob + 1) * 128],
                        xT_sb[:, ib, :],
                        start=(ib == 0 and ob % 2 == 0),
                        stop=(ib == 7 and ob % 2 == 1),
                    )
            for ob in range(8):
                src_ps = qps[ob // 4][:, (ob % 4) * SQ:(ob % 4 + 1) * SQ]
                nc.scalar.activation(
                    qT_sb[0:64, 2 * ob, :], src_ps[0:64, :],
                    AF.Identity, bias=bq_sb[0:64, ob:ob + 1], scale=1.0,
                )
                stg = stagep.tile([128, SQ], F16, tag="stg")
                nc.scalar.activation(
                    stg[64:128, :], src_ps[64:128, :],
                    AF.Identity, bias=bq_sb[64:128, ob:ob + 1], scale=1.0,
                )
                nc.sync.dma_start(qT_sb[0:64, 2 * ob + 1, :], stg[64:128, :])

            # ---- cache chunk loop ----
            wv_view = None
            wc_view = None
            wd_view = None
            for c in range(NCH):
                hot = c < HOT_NCH
                c0 = c * CH
                if c == 0:
                    vw_flat = vwp.tile([128, 8 * HID], F16, tag="vw")
                    wv_view = vw_flat.rearrange("p (ib o) -> p ib o", ib=8)
                    nc.sync.dma_start(
                        wv_view,
                        reg("Wv").rearrange("(ib p o) -> p ib o", p=128, o=HID))
                if c == HOT_NCH:
                    vw_flat = vwp.tile([128, 8 * HID], F16, tag="vw")
                    wc_view = vw_flat[:, 0:8 * COMP].rearrange(
                        "p (ib o) -> p ib o", ib=8)
                    nc.sync.dma_start(
                        wc_view,
                        reg("Wc").rearrange("(ib p o) -> p ib o", p=128, o=COMP))
                    wd_view = vw_flat[:, 8 * COMP:8 * COMP + 4 * HID].rearrange(
                        "p (ib o) -> p ib o", ib=4)
                    nc.sync.dma_start(
                        wd_view,
                        reg("Wd").rearrange("(ib p o) -> p ib o", p=128, o=HID))

                ktc = krawp.tile([128, 8, CH], F16, tag="ktc")
                nc.sync.dma_start(ktc, keys_v[:, :, c0:c0 + CH])
                vtc = krawp.tile([128, 8, CH], F16, tag="vtc")
                vsrc = vhot_v[:, :, c0:c0 + CH] if hot else \
                    vcold_v[:, :, c0 - HOT:c0 - HOT + CH]
                nc.sync.dma_start(vtc, vsrc)

                # -- K projection: kT[o, c] = Wk.T @ keysT_chunk --
                kt = kprojp.tile([64, NH, CH], F16, tag="kt")
                for ob in range(8):
                    ps = pproj.tile([128, 512], F32, tag="pp")
                    for ib in range(8):
                        nc.tensor.matmul(
                            ps,
                            wk_sb[:, ib, ob * 128:(ob + 1) * 128],
                            ktc[:, ib, :],
                            start=(ib == 0), stop=(ib == 7),
                        )
                    if ob % 2 == 0:
                        nc.scalar.copy(kt[0:64, ob, :], ps[0:64, :])
                        stg = stagep.tile([128, CH], F16, tag="stgk")
                        nc.vector.tensor_copy(stg[64:128, :], ps[64:128, :])
                    else:
                        nc.vector.tensor_copy(kt[0:64, ob, :], ps[0:64, :])
                        stg = stagep.tile([128, CH], F16, tag="stgk")
                        nc.scalar.copy(stg[64:128, :], ps[64:128, :])
                    nc.sync.dma_start(kt[0:64, ob + 8, :], stg[64:128, :])

                # -- V projection into vext [c, 16*(64+1)] (ones col per head) --
                vext_t = vextp.tile([128, NCB, NH * (HD + 1)], F16, tag="vext")
                if hot:
                    for cb in range(NCB):
                        for oc in range(2):
                            ps = pproj.tile([128, 512], F32, tag="pp")
                            for ib in range(8):
                                nc.tensor.matmul(
                                    ps,
                                    vtc[:, ib, cb * 128:(cb + 1) * 128],
                                    wv_view[:, ib, oc * 512:(oc + 1) * 512],
                                    start=(ib == 0), stop=(ib == 7),
                                )
                            dst = vext_t[:, cb, oc * 520:(oc + 1) * 520].rearrange(
                                "p (h e) -> p h e", h=8)[:, :, 0:HD]
                            nc.vector.tensor_copy(
                                dst, ps[:, :].rearrange("p (h e) -> p h e", e=HD))
                else:
                    # compress: cT[o', c] = Wc.T @ valuesT_chunk (+bc)
                    ct = ctp.tile([128, 4, CH], F16, tag="ct")
                    for obq in range(4):
                        ps = pproj.tile([128, 512], F32, tag="pp")
                        for ib in range(8):
                            nc.tensor.matmul(
                                ps,
                                wc_view[:, ib, obq * 128:(obq + 1) * 128],
                                vtc[:, ib, :],
                                start=(ib == 0), stop=(ib == 7),
                            )
                        nc.scalar.activation(
                            ct[:, obq, :], ps,
                            AF.Identity, bias=bc_sb[:, obq:obq + 1], scale=1.0,
                        )
                    # decompress: v[c, o] = cT.T @ Wd
                    for cb in range(NCB):
                        for oc in range(2):
                            ps = pproj.tile([128, 512], F32, tag="pp")
                            for ibq in range(4):
                                nc.tensor.matmul(
                                    ps,
                                    ct[:, ibq, cb * 128:(cb + 1) * 128],
                                    wd_view[:, ibq, oc * 512:(oc + 1) * 512],
                                    start=(ibq == 0), stop=(ibq == 3),
                                )
                            dst = vext_t[:, cb, oc * 520:(oc + 1) * 520].rearrange(
                                "p (h e) -> p h e", h=8)[:, :, 0:HD]
                            nc.vector.tensor_copy(
                                dst, ps[:, :].rearrange("p (h e) -> p h e", e=HD))
                nc.vector.memset(
                    vext_t.rearrange("p cb (h e) -> p cb h e", e=HD + 1)[:, :, :, HD:HD + 1],
                    1.0)

                # -- attention for this chunk --
                for hg in range(4):
                    e_ts = []
                    for cb in range(NCB):
                        g = c * NCB + cb
                        stp = pst.tile([128, 4 * SQ], F32, tag="st")
                        for hh in range(4):
                            h = hg * 4 + hh
                            ki = (h // 2) if h % 2 == 0 else (h // 2 + 8)
                            nc.tensor.matmul(
                                stp[:, hh * SQ:(hh + 1) * SQ],
                                kt[0:64, ki, cb * 128:(cb + 1) * 128],
                                qT_sb[0:64, h, :],
                                start=(hh % 2 == 0), stop=(hh % 2 == 1),
                            )
                        e_t = epool.tile([128, 4, SQ], F16, tag="e")
                        nc.scalar.activation(
                            e_t, stp[:, :].rearrange("p (a b) -> p a b", a=4),
                            AF.Exp, bias=biasc_sb[:, g:g + 1], scale=0.125,
                        )
                        e_ts.append(e_t)
                    for pr in range(2):
                        pa = pacc.tile([128, 2 * SQ], F32, tag="pa")
                        for cb in range(NCB):
                            for sub in range(2):
                                h = hg * 4 + pr * 2 + sub
                                nc.tensor.matmul(
                                    pa[0:65, sub * SQ:(sub + 1) * SQ],
                                    vext_t[:, cb, h * 65:h * 65 + 65],
                                    e_ts[cb][:, pr * 2 + sub, :],
                                    start=(cb == 0 and sub == 0),
                                    stop=(cb == NCB - 1 and sub == 1),
                                )
                        h0 = hg * 4 + pr * 2
                        dst = acc_sb[0:65, h0:h0 + 2, :]
                        src = pa[0:65, :].rearrange("p (a b) -> p a b", a=2)
                        if c == 0 or c == HOT_NCH:
                            nc.vector.tensor_copy(dst, src)
                        else:
                            nc.vector.tensor_add(dst, dst, src)

                # -- per-tier softmax division at tier end --
                if c == HOT_NCH - 1 or c == NCH - 1:
                    first_tier = c == HOT_NCH - 1
                    nc.vector.reciprocal(acc_sb[64:65, :, :], acc_sb[64:65, :, :])
                    nc.sync.dma_start(
                        lscr[0:1, :],
                        acc_sb[64:65, :, :].rearrange("p a b -> p (a b)"))
                    for h in range(NH):
                        if h % 8 == 0:
                            nc.sync.dma_start(
                                lbc_sb,
                                lscr[0:1, (h // 8) * 8 * SQ:(h // 8 + 1) * 8 * SQ]
                                .to_broadcast([64, 8 * SQ]).rearrange(
                                    "p (a b) -> p a b", a=8))
                        num = acc_sb[0:64, h, :]
                        rc = lbc_sb[0:64, h % 8, :]
                        dst = aoT_sb[(h % 2) * 64:(h % 2) * 64 + 64, h // 2, :]
                        if first_tier:
                            if h % 2 == 0:
                                nc.vector.tensor_mul(dst, num, rc)
                            else:
                                tmp = epool.tile([128, 4, SQ], F32, tag="dtmp", bufs=2)
                                nc.vector.tensor_mul(tmp[0:64, 0, :], num, rc)
                                nc.sync.dma_start(dst, tmp[0:64, 0, :])
                        else:
                            tmp = epool.tile([128, 4, SQ], F32, tag="dtmp", bufs=2)
                            nc.vector.tensor_mul(tmp[0:64, 0, :], num, rc)
                            if h % 2 == 0:
                                nc.vector.tensor_add(dst, dst, tmp[0:64, 0, :])
                            else:
                                tmp2 = epool.tile([128, 4, SQ], F32, tag="dtmp", bufs=2)
                                nc.sync.dma_start(
                                    tmp2[64:128, 0, :], tmp[0:64, 0, :])
                                nc.vector.tensor_add(dst, dst, tmp2[64:128, 0, :])

            # ---- output projection y = aoT.T @ Wo + cvec (fp16), layernorm ----
            nc.vector.tensor_copy(aoT16_sb, aoT_sb)
            yps = [pst.tile([128, 4 * SQ], F32, tag="st", name=f"yps{i}") for i in range(2)]
            for ib in range(8):
                wo_strip = wrowp.tile([128, HID], F16, tag="wo")
                nc.sync.dma_start(wo_strip, wo_v[ib * 128:(ib + 1) * 128, :])
                for sblk in range(2):
                    for oc in range(2):
                        nc.tensor.matmul(
                            yps[sblk][:, oc * 512:(oc + 1) * 512],
                            aoT16_sb[:, ib, sblk * 128:(sblk + 1) * 128],
                            wo_strip[:, oc * 512:(oc + 1) * 512],
                            start=(ib == 0), stop=False,
                        )
            for sblk in range(2):
                for oc in range(2):
                    nc.tensor.matmul(
                        yps[sblk][:, oc * 512:(oc + 1) * 512],
                        ones_sb[0:1, 0:128],
                        cvec_sb[0:1, oc * 512:(oc + 1) * 512],
                        start=False, stop=True,
                    )

            gb_t = gbpool.tile([128, 2 * HID], F32, tag="gb")
            go, _ = OFF["gamma"]
            for gch in range(4):
                gst = stagep.tile([128, CH], F16, tag="stgk")
                nc.sync.dma_start(
                    gst, blob[go + gch * CH:go + (gch + 1) * CH]
                    .unsqueeze(0).to_broadcast([128, CH]))
                nc.vector.tensor_copy(gb_t[:, gch * CH:(gch + 1) * CH], gst)

            for sblk in range(2):
                y_sb = ypool.tile([128, HID], F32, tag="y")
                nc.scalar.copy(y_sb, yps[sblk][:, :])
                stats = ypool.tile([128, 2, 6], F32, tag="stats")
                for sub in range(2):
                    nc.vector.bn_stats(
                        stats[:, sub, :], y_sb[:, sub * 512:(sub + 1) * 512])
                mv = ypool.tile([128, 2], F32, tag="mv")
                nc.vector.bn_aggr(mv, stats)
                rstd = ypool.tile([128, 1], F32, tag="rstd")
                nc.scalar.activation(
                    rstd, mv[:, 1:2], AF.Sqrt, bias=eps_sb[:, 0:1], scale=1.0)
                nc.vector.reciprocal(rstd, rstd)
                nc.vector.tensor_scalar(
                    y_sb, y_sb, mv[:, 0:1], rstd,
                    op0=mybir.AluOpType.subtract, op1=mybir.AluOpType.mult)
                nc.vector.tensor_mul(y_sb, y_sb, gb_t[:, 0:HID])
                nc.vector.tensor_add(y_sb, y_sb, gb_t[:, HID:2 * HID])
                nc.sync.dma_start(y_out[sblk * 128:(sblk + 1) * 128, :], y_sb)

    if split_for_hw:
        split_waits(nc)
    return nc


_NC_CACHE = None


def _get_nc():
    global _NC_CACHE
    if _NC_CACHE is None:
        _NC_CACHE = build_nc()
    return _NC_CACHE


def _prep_inputs(inputs):
    f32 = lambda a: np.asarray(a, dtype=np.float32)
    f16 = lambda a: np.ascontiguousarray(np.asarray(a, dtype=np.float32).astype(np.float16))
    x = np.asarray(inputs["inputs"], dtype=np.float32).reshape(B * S, HID)
    keys = np.concatenate(
        [np.asarray(inputs["hot_keys"], np.float32),
         np.asarray(inputs["cold_keys"], np.float32)], axis=0)
    biasc = np.concatenate([
        -0.1 * f32(inputs["hot_age"]) + 0.05 * f32(inputs["hot_access"]),
        -0.1 * f32(inputs["cold_age"]) + 0.05 * f32(inputs["cold_access"]),
    ])
    bv = f32(inputs["bv"])
    bd = f32(inputs["bd"])
    bo = f32(inputs["bo"])
    Wo = f32(inputs["Wo"])
    cvec = (bv + bd) @ Wo + 2.0 * bo
    shared_parts = [
        f16(keys.T).ravel(),
        f16(np.asarray(inputs["hot_values"], np.float32).T).ravel(),
        f16(np.asarray(inputs["cold_values"], np.float32).T).ravel(),
        f16(inputs["Wq"]).ravel(),
        f16(inputs["Wk"]).ravel(),
        f16(inputs["Wv"]).ravel(),
        f16(inputs["Wc"]).ravel(),
        f16(inputs["Wd"]).ravel(),
        f16(Wo).ravel(),
        f16(inputs["bq"]).ravel(),
        f16(inputs["bc"]).ravel(),
        biasc.astype(np.float16).ravel(),
        cvec.astype(np.float16).ravel(),
        f16(inputs["gamma"]).ravel(),
        f16(inputs["beta"]).ravel(),
    ]
    shared = np.concatenate(shared_parts)
    nx = HID * SQ
    assert shared.size == BLOB_N - nx
    xT16 = np.asarray(x.T, np.float32).astype(np.float16)
    blobs = np.empty((NCORES, BLOB_N), np.float16)
    blobs[:, nx:] = shared
    for i in range(NCORES):
        blobs[i, :nx] = np.ascontiguousarray(
            xT16[:, i * SQ:(i + 1) * SQ]).ravel()
    return [{"blob": blobs[i]} for i in range(NCORES)]


def _run(inputs, trace=False):
    from concourse.bass_utils import run_bass_kernel_spmd

    nc = _get_nc()
    in_maps = _prep_inputs(inputs)
    res = run_bass_kernel_spmd(
        nc, in_maps, core_ids=list(range(NCORES)), trace=trace)
    y = np.concatenate(
        [res.results[i]["y_shard"] for i in range(NCORES)], axis=0)
    return y.reshape(B, S, HID), res


def kernel(**inputs):
    y, _ = _run(inputs, trace=False)
    return y


def make_test_inputs(seed=0):
    rng = np.random.default_rng(seed)
    std = 0.02
    return {
        "inputs": rng.standard_normal((B, S, HID)).astype(np.float32),
        "hot_keys": (std * rng.standard_normal((HOT, HID))).astype(np.float32),
        "hot_values": (std * rng.standard_normal((HOT, HID))).astype(np.float32),
        "hot_age": np.abs(rng.standard_normal(HOT)).astype(np.float32),
        "hot_access": np.abs(rng.standard_normal(HOT)).astype(np.float32),
        "cold_keys": (std * rng.standard_normal((COLD, HID))).astype(np.float32),
        "cold_values": (std * rng.standard_normal((COLD, HID))).astype(np.float32),
        "cold_age": np.abs(rng.standard_normal(COLD)).astype(np.float32),
        "cold_access": np.abs(rng.standard_normal(COLD)).astype(np.float32),
        "Wq": (std * rng.standard_normal((HID, HID))).astype(np.float32),
        "bq": (0.01 * rng.standard_normal(HID)).astype(np.float32),
        "Wk": (std * rng.standard_normal((HID, HID))).astype(np.float32),
        "bk": (0.01 * rng.standard_normal(HID)).astype(np.float32),
        "Wv": (std * rng.standard_normal((HID, HID))).astype(np.float32),
        "bv": (0.01 * rng.standard_normal(HID)).astype(np.float32),
        "Wo": (std * rng.standard_normal((HID, HID))).astype(np.float32),
        "bo": (0.01 * rng.standard_normal(HID)).astype(np.float32),
        "Wc": ((1.0 / np.sqrt(HID)) * rng.standard_normal((HID, COMP))).astype(np.float32),
        "bc": (0.01 * rng.standard_normal(COMP)).astype(np.float32),
        "Wd": ((1.0 / np.sqrt(COMP)) * rng.standard_normal((COMP, HID))).astype(np.float32),
        "bd": (0.01 * rng.standard_normal(HID)).astype(np.float32),
        "gamma": (1.0 + 0.1 * rng.standard_normal(HID)).astype(np.float32),
        "beta": (0.1 * rng.standard_normal(HID)).astype(np.float32),
    }


def np_reference(inp):
    x = np.asarray(inp["inputs"], np.float64).reshape(B * S, HID)
    q = x @ inp["Wq"] + inp["bq"]
    keys = np.concatenate([inp["hot_keys"], inp["cold_keys"]]).astype(np.float64)
    k = keys @ inp["Wk"] + inp["bk"]
    hot_v = inp["hot_values"].astype(np.float64) @ inp["Wv"] + inp["bv"]
    cold_v = (inp["cold_values"].astype(np.float64) @ inp["Wc"] + inp["bc"]) \
        @ inp["Wd"] + inp["bd"]
    biasv = np.concatenate([
        -0.1 * inp["hot_age"] + 0.05 * inp["hot_access"],
        -0.1 * inp["cold_age"] + 0.05 * inp["cold_access"]]).astype(np.float64)
    qh = q.reshape(B * S, NH, HD)
    kh = k.reshape(CACHE, NH, HD)
    out = np.zeros((B * S, NH, HD))
    for lo, hi, v in [(0, HOT, hot_v), (HOT, CACHE, cold_v)]:
        sc = np.einsum("snd,cnd->snc", qh, kh[lo:hi]) / np.sqrt(HD)
        sc = sc + biasv[lo:hi][None, None, :]
        a = np.exp(sc)
        a /= a.sum(-1, keepdims=True)
        out += np.einsum("snc,cnd->snd", a, v.reshape(hi - lo, NH, HD))
    xx = out.reshape(B * S, HID) @ inp["Wo"] + 2 * inp["bo"]
    mu = xx.mean(-1, keepdims=True)
    var = ((xx - mu) ** 2).mean(-1, keepdims=True)
    y = (xx - mu) / np.sqrt(var + EPS) * inp["gamma"] + inp["beta"]
    return y.reshape(B, S, HID)


if __name__ == "__main__":
    # single-core CoreSim smoke test against the numpy reference
    from concourse.bass_interp import CoreSim

    inputs = make_test_inputs()
    expected = np_reference(inputs)

    nc = build_nc(split_for_hw=False)
    in_maps = _prep_inputs(inputs)
    sim = CoreSim(nc)
    for kname, v in in_maps[0].items():
        sim.tensor(kname)[:] = v
    sim.simulate(check_with_hw=False)
    got = np.array(sim.tensor("y_shard"))
    exp0 = expected.reshape(B * S, HID)[0:SQ]
    err = np.abs(got - exp0)
    denom = np.abs(exp0).max()
    print(f"core0 absmax_err={err.max():.3e} relmax={err.max() / denom:.3e} "
          f"mean={err.mean():.3e}")
